# revision 5
# baseline (speedup 1.0000x reference)
"""3D Haar DWT (2x2x2 stride-2) on 8 Trainium2 NeuronCores.

Input  x: (2, 32, 64, 128, 128) fp32, kernels: (8, 2, 2, 2) fp32 (fixed Haar taps).
Output (low, highs): low (2, 32, 32, 64, 64), highs (2, 32, 7, 32, 64, 64).

Sharding: pure data parallel over the 64 (b, c) channel-planes -> 8 channels
per core.  Per core the transform is computed separably:
  - stage X (w butterfly)  : DVE tensor_add/tensor_sub on stride-2 free-dim APs
  - stage Y (h butterfly)  : DVE tensor_add/tensor_sub on free-dim APs
  - stage Z (d butterfly + global 1/(2*sqrt(2)) scale): one 128x128 fp32 matmul
    per 512-wide chunk.  Partition dim is (d-parity, channel-in-pair, d'), so
    the Z weight is the block matrix [[I64, I64], [I64, -I64]] * tz/2.
  - ScalarE copies PSUM->SBUF, then per-subband DMA-out with 4 KB granules.

All HBM traffic is contiguous in >=4 KiB runs (load granule: 16 KiB).
"""

import numpy as np

# Per-core problem geometry (hardcoded; the harness always passes the full
# (2, 32, 64, 128, 128) input).
B, C, D, H, W = 2, 32, 64, 128, 128
NCORES = 8
CC = (B * C) // NCORES          # 8 channel-planes per core
D2, H2, W2 = D // 2, H // 2, W // 2
NCPAIR = CC // 2                # 4 channel pairs per core
NHCHUNK = 4                     # h chunks of 32 lines
HCH = H // NHCHUNK              # 32 h lines per chunk
FREE = HCH * W                  # 4096 fp32 per partition per tile

_CACHE = {}


def _build_module():
    import concourse.bacc as bacc
    import concourse.mybir as mybir
    import concourse.tile as tile

    f32 = mybir.dt.float32
    nc = bacc.Bacc(None, target_bir_lowering=False)

    x = nc.dram_tensor("x", [CC, D, H, W], f32, kind="ExternalInput")
    wz = nc.dram_tensor("wz", [128, 128], f32, kind="ExternalInput")
    low = nc.dram_tensor("low", [CC, D2, H2, W2], f32, kind="ExternalOutput")
    high = nc.dram_tensor("high", [CC, 7, D2, H2, W2], f32, kind="ExternalOutput")

    OFREE = H2 * W2 * 4            # 16384: full (sy, sx, h2, w') per partition

    with tile.TileContext(nc) as tc:
        with (
            tc.tile_pool(name="wpool", bufs=1) as wpool,
            tc.tile_pool(name="io", bufs=3) as io,
            tc.tile_pool(name="mid", bufs=2) as mid,
            tc.tile_pool(name="obuf", bufs=1) as obuf,
            tc.tile_pool(name="psum", bufs=8, space="PSUM") as psum,
        ):
            w_sb = wpool.tile([128, 128], f32)
            nc.sync.dma_start(out=w_sb[:, :], in_=wz[:, :])

            for cp in range(NCPAIR):
                # Full-H output accumulator: free = (sy, sx, h2:64, w':64).
                # Collecting all 4 h-chunks before storing gives 16 KiB
                # contiguous store descriptors (4 KiB descriptors pay ~74 ns
                # fixed cost each -> only ~16 GB/s per SDMA engine).
                ot = obuf.tile([128, OFREE], f32, tag="ot")
                for hc in range(NHCHUNK):
                    # ---- load: partition p = dp*4 + i*2 + u, free = (h, w).
                    # dp (32) is the OUTERMOST dram-AP dim: HWDGE assigns
                    # descriptors to SDMA engines by outer-dim index, so an
                    # outer dim of 32 spreads across all 16 engines (outer=2
                    # degenerates to 2 engines at ~50 GB/s).
                    xin = io.tile([128, FREE], f32, tag="xin")
                    src = x[2 * cp:2 * cp + 2, :, hc * HCH:(hc + 1) * HCH, :]
                    src = src.rearrange("c (dp i) h w -> dp i c (h w)", i=2)
                    nc.sync.dma_start(out=xin[:, :], in_=src)

                    # ---- stage X: w butterfly -> xt free = (sx, h, w')
                    xt = mid.tile([128, FREE], f32, tag="xt")
                    xv = xin[:, :].rearrange("p (f two) -> p f two", two=2)
                    xe, xo = xv[:, :, 0], xv[:, :, 1]
                    nc.vector.tensor_add(out=xt[:, 0:FREE // 2], in0=xe, in1=xo)
                    nc.vector.tensor_sub(out=xt[:, FREE // 2:FREE], in0=xe, in1=xo)

                    # ---- stage Y: h butterfly -> yt free = (sy, sx, h2, w')
                    yt = mid.tile([128, FREE], f32, tag="yt")
                    xtv = xt[:, :].rearrange(
                        "p (sx h2 two wp) -> p sx h2 two wp", sx=2, h2=HCH // 2, two=2
                    )
                    ye, yo = xtv[:, :, :, 0, :], xtv[:, :, :, 1, :]
                    ytv = yt[:, :].rearrange(
                        "p (sy sx h2 wp) -> p sy sx h2 wp", sy=2, sx=2, h2=HCH // 2
                    )
                    nc.vector.tensor_add(out=ytv[:, 0], in0=ye, in1=yo)
                    nc.vector.tensor_sub(out=ytv[:, 1], in0=ye, in1=yo)

                    # ---- stage Z: d butterfly + scale, one matmul per 512
                    # cols; ScalarE drains PSUM into the (sy, sx, h2g, w')
                    # slot of the full-H accumulator.
                    for n in range(FREE // 512):
                        ps = psum.tile([128, 512], f32, tag="ps")
                        nc.tensor.matmul(
                            ps[:, :], w_sb[:, :], yt[:, n * 512:(n + 1) * 512],
                            start=True, stop=True,
                        )
                        sy_, sx_, hh = n // 4, (n // 2) % 2, n % 2
                        off = sy_ * (OFREE // 2) + sx_ * (OFREE // 4) \
                            + hc * 1024 + hh * 512
                        nc.scalar.copy(out=ot[:, off:off + 512], in_=ps[:, :])

                # ---- store: one DMA per subband covering all of (h', w'):
                # 16 KiB contiguous per (dp, c) descriptor, outer dim 32,
                # issued from the ACT HWDGE ring so loads (SP ring) overlap.
                for sz in range(2):
                    for sy in range(2):
                        for sx in range(2):
                            s = sz * 4 + sy * 2 + sx
                            fo = sy * (OFREE // 2) + sx * (OFREE // 4)
                            sb = ot[sz * 64:(sz + 1) * 64, fo:fo + OFREE // 4]
                            if s == 0:
                                dst = low[2 * cp:2 * cp + 2]
                            else:
                                dst = high[2 * cp:2 * cp + 2, s - 1]
                            dst = dst.rearrange("c dp h w -> dp c (h w)")
                            nc.scalar.dma_start(out=dst, in_=sb)

    nc.finalize()
    return nc


def _zweights(kernels: np.ndarray) -> np.ndarray:
    """Stage-Z weight: wz[k=(dp,i,u), m=(sz,dp,u)] = tz_sz[i] / 2 on the
    (dp,u) diagonal.  kernels[4*sz, i, 0, 0] = tz_sz[i] * ty0[0] * tx0[0]
    = tz_sz[i]/2 exactly as the reference computed it in fp32."""
    wz = np.zeros((128, 128), np.float32)
    dp = np.arange(32)
    for i in range(2):
        for u in range(2):
            for sz in range(2):
                wz[dp * 4 + i * 2 + u, sz * 64 + dp * 2 + u] = np.float32(
                    kernels[4 * sz, i, 0, 0]
                )
    return wz


def kernel(x, kernels):
    from concourse.bass_utils import run_bass_kernel_spmd

    x = np.asarray(x, dtype=np.float32)
    kernels = np.asarray(kernels, dtype=np.float32)
    assert x.shape == (B, C, D, H, W), x.shape

    if "nc" not in _CACHE:
        _CACHE["nc"] = _build_module()
    nc = _CACHE["nc"]

    wz = _zweights(kernels)
    xf = x.reshape(B * C, D, H, W)
    in_maps = [
        {"x": np.ascontiguousarray(xf[k * CC:(k + 1) * CC]), "wz": wz}
        for k in range(NCORES)
    ]
    res = run_bass_kernel_spmd(nc, in_maps, core_ids=list(range(NCORES)))

    low = np.concatenate([r["low"] for r in res.results], axis=0)
    high = np.concatenate([r["high"] for r in res.results], axis=0)
    low = low.reshape(B, C, D2, H2, W2)
    high = high.reshape(B, C, 7, D2, H2, W2)
    return low, high


# revision 6
# speedup vs baseline: 1.1317x; 1.1317x over previous
"""3D Haar DWT (2x2x2 stride-2) on 8 Trainium2 NeuronCores.

Input  x: (2, 32, 64, 128, 128) fp32, kernels: (8, 2, 2, 2) fp32 (fixed Haar taps).
Output (low, highs): low (2, 32, 32, 64, 64), highs (2, 32, 7, 32, 64, 64).

Sharding: pure data parallel over the 64 (b, c) channel-planes -> 8 channels
per core.  Per core the transform is computed separably:
  - stage X (w butterfly)  : DVE tensor_add/tensor_sub on stride-2 free-dim APs
  - stage Y (h butterfly)  : DVE tensor_add/tensor_sub on free-dim APs
  - stage Z (d butterfly + global 1/(2*sqrt(2)) scale): one 128x128 fp32 matmul
    per 512-wide chunk.  Partition dim is (d-parity, channel-in-pair, d'), so
    the Z weight is the block matrix [[I64, I64], [I64, -I64]] * tz/2.
  - ScalarE copies PSUM->SBUF, then per-subband DMA-out with 4 KB granules.

All HBM traffic is contiguous in >=4 KiB runs (load granule: 16 KiB).
"""

import numpy as np

# Per-core problem geometry (hardcoded; the harness always passes the full
# (2, 32, 64, 128, 128) input).
B, C, D, H, W = 2, 32, 64, 128, 128
NCORES = 8
CC = (B * C) // NCORES          # 8 channel-planes per core
D2, H2, W2 = D // 2, H // 2, W // 2
NCPAIR = CC // 2                # 4 channel pairs per core
NHCHUNK = 4                     # h chunks of 32 lines
HCH = H // NHCHUNK              # 32 h lines per chunk
FREE = HCH * W                  # 4096 fp32 per partition per tile

_CACHE = {}


def _build_module():
    import concourse.bacc as bacc
    import concourse.mybir as mybir
    import concourse.tile as tile

    f32 = mybir.dt.float32
    nc = bacc.Bacc(None, target_bir_lowering=False)

    x = nc.dram_tensor("x", [CC, D, H, W], f32, kind="ExternalInput")
    wz = nc.dram_tensor("wz", [128, 128], f32, kind="ExternalInput")
    low = nc.dram_tensor("low", [CC, D2, H2, W2], f32, kind="ExternalOutput")
    high = nc.dram_tensor("high", [CC, 7, D2, H2, W2], f32, kind="ExternalOutput")

    with tile.TileContext(nc) as tc:
        with (
            tc.tile_pool(name="wpool", bufs=1) as wpool,
            tc.tile_pool(name="io", bufs=3) as io,
            tc.tile_pool(name="mid", bufs=2) as mid,
            tc.tile_pool(name="obuf", bufs=4) as obuf,
            tc.tile_pool(name="psum", bufs=8, space="PSUM") as psum,
        ):
            w_sb = wpool.tile([128, 128], f32)
            nc.sync.dma_start(out=w_sb[:, :], in_=wz[:, :])

            for cp in range(NCPAIR):
                for hc in range(NHCHUNK):
                    # ---- load: partition p = dp*4 + i*2 + u, free = (h, w).
                    # dp (32) is the OUTERMOST dram-AP dim: HWDGE assigns
                    # descriptors to SDMA engines by outer-dim index, so an
                    # outer dim of 32 spreads across all 16 engines (outer=2
                    # degenerates to 2 engines at ~50 GB/s).
                    xin = io.tile([128, FREE], f32, tag="xin")
                    src = x[2 * cp:2 * cp + 2, :, hc * HCH:(hc + 1) * HCH, :]
                    src = src.rearrange("c (dp i) h w -> dp i c (h w)", i=2)
                    nc.sync.dma_start(out=xin[:, :], in_=src)

                    # ---- stage X: w butterfly -> xt free = (sx, h, w')
                    xt = mid.tile([128, FREE], f32, tag="xt")
                    xv = xin[:, :].rearrange("p (f two) -> p f two", two=2)
                    xe, xo = xv[:, :, 0], xv[:, :, 1]
                    nc.vector.tensor_add(out=xt[:, 0:FREE // 2], in0=xe, in1=xo)
                    nc.vector.tensor_sub(out=xt[:, FREE // 2:FREE], in0=xe, in1=xo)

                    # ---- stage Y: h butterfly -> yt free = (sy, sx, h2, w')
                    yt = mid.tile([128, FREE], f32, tag="yt")
                    xtv = xt[:, :].rearrange(
                        "p (sx h2 two wp) -> p sx h2 two wp", sx=2, h2=HCH // 2, two=2
                    )
                    ye, yo = xtv[:, :, :, 0, :], xtv[:, :, :, 1, :]
                    ytv = yt[:, :].rearrange(
                        "p (sy sx h2 wp) -> p sy sx h2 wp", sy=2, sx=2, h2=HCH // 2
                    )
                    nc.vector.tensor_add(out=ytv[:, 0], in0=ye, in1=yo)
                    nc.vector.tensor_sub(out=ytv[:, 1], in0=ye, in1=yo)

                    # ---- stage Z: d butterfly + scale, one matmul per 512
                    # cols; ScalarE drains PSUM -> ot.
                    ot = obuf.tile([128, FREE], f32, tag="ot")
                    for n in range(FREE // 512):
                        ps = psum.tile([128, 512], f32, tag="ps")
                        nc.tensor.matmul(
                            ps[:, :], w_sb[:, :], yt[:, n * 512:(n + 1) * 512],
                            start=True, stop=True,
                        )
                        nc.scalar.copy(out=ot[:, n * 512:(n + 1) * 512], in_=ps[:, :])

                    # ---- store: one DMA per subband, alternating between the
                    # two HWDGE rings (SP & ACT) so descriptor generation never
                    # starves the write stream (writes cap at ~215 GB/s and are
                    # the long pole; any write-idle hole is lost wall time).
                    h2lo = hc * (HCH // 2)
                    h2hi = h2lo + HCH // 2
                    for sz in range(2):
                        for sy in range(2):
                            for sx in range(2):
                                s = sz * 4 + sy * 2 + sx
                                fo = sy * (FREE // 2) + sx * (FREE // 4)
                                sb = ot[sz * 64:(sz + 1) * 64, fo:fo + FREE // 4]
                                if s == 0:
                                    dst = low[2 * cp:2 * cp + 2, :, h2lo:h2hi, :]
                                else:
                                    dst = high[2 * cp:2 * cp + 2, s - 1, :, h2lo:h2hi, :]
                                dst = dst.rearrange("c dp h w -> dp c (h w)")
                                eng = nc.scalar if s % 2 == 0 else nc.sync
                                eng.dma_start(out=dst, in_=sb)

    nc.finalize()
    return nc


def _zweights(kernels: np.ndarray) -> np.ndarray:
    """Stage-Z weight: wz[k=(dp,i,u), m=(sz,dp,u)] = tz_sz[i] / 2 on the
    (dp,u) diagonal.  kernels[4*sz, i, 0, 0] = tz_sz[i] * ty0[0] * tx0[0]
    = tz_sz[i]/2 exactly as the reference computed it in fp32."""
    wz = np.zeros((128, 128), np.float32)
    dp = np.arange(32)
    for i in range(2):
        for u in range(2):
            for sz in range(2):
                wz[dp * 4 + i * 2 + u, sz * 64 + dp * 2 + u] = np.float32(
                    kernels[4 * sz, i, 0, 0]
                )
    return wz


def kernel(x, kernels):
    from concourse.bass_utils import run_bass_kernel_spmd

    x = np.asarray(x, dtype=np.float32)
    kernels = np.asarray(kernels, dtype=np.float32)
    assert x.shape == (B, C, D, H, W), x.shape

    if "nc" not in _CACHE:
        _CACHE["nc"] = _build_module()
    nc = _CACHE["nc"]

    wz = _zweights(kernels)
    xf = x.reshape(B * C, D, H, W)
    in_maps = [
        {"x": np.ascontiguousarray(xf[k * CC:(k + 1) * CC]), "wz": wz}
        for k in range(NCORES)
    ]
    res = run_bass_kernel_spmd(nc, in_maps, core_ids=list(range(NCORES)))

    low = np.concatenate([r["low"] for r in res.results], axis=0)
    high = np.concatenate([r["high"] for r in res.results], axis=0)
    low = low.reshape(B, C, D2, H2, W2)
    high = high.reshape(B, C, 7, D2, H2, W2)
    return low, high


# revision 8
# speedup vs baseline: 1.2100x; 1.0692x over previous
"""3D Haar DWT (2x2x2 stride-2) on 8 Trainium2 NeuronCores.

Input  x: (2, 32, 64, 128, 128) fp32, kernels: (8, 2, 2, 2) fp32 (fixed Haar taps).
Output (low, highs): low (2, 32, 32, 64, 64), highs (2, 32, 7, 32, 64, 64).

Sharding: pure data parallel over the 64 (b, c) channel-planes -> 8 channels
per core.  Per core the transform is computed separably:
  - stage X (w butterfly)  : DVE tensor_add/tensor_sub on stride-2 free-dim APs
  - stage Y (h butterfly)  : DVE tensor_add/tensor_sub on free-dim APs
  - stage Z (d butterfly + global 1/(2*sqrt(2)) scale): one 128x128 fp32 matmul
    per 512-wide chunk.  Partition dim is (d-parity, channel-in-pair, d'), so
    the Z weight is the block matrix [[I64, I64], [I64, -I64]] * tz/2.
  - ScalarE copies PSUM->SBUF, then per-subband DMA-out with 4 KB granules.

All HBM traffic is contiguous in >=4 KiB runs (load granule: 16 KiB).
"""

import numpy as np

# Per-core problem geometry (hardcoded; the harness always passes the full
# (2, 32, 64, 128, 128) input).
B, C, D, H, W = 2, 32, 64, 128, 128
NCORES = 8
CC = (B * C) // NCORES          # 8 channel-planes per core
D2, H2, W2 = D // 2, H // 2, W // 2
NCPAIR = CC // 2                # 4 channel pairs per core
NHCHUNK = 4                     # h chunks of 32 lines
HCH = H // NHCHUNK              # 32 h lines per chunk
FREE = HCH * W                  # 4096 fp32 per partition per tile

_CACHE = {}


def _build_module():
    import concourse.bacc as bacc
    import concourse.mybir as mybir
    import concourse.tile as tile

    f32 = mybir.dt.float32
    nc = bacc.Bacc(None, target_bir_lowering=False)

    x = nc.dram_tensor("x", [CC, D, H, W], f32, kind="ExternalInput")
    wz = nc.dram_tensor("wz", [128, 128], f32, kind="ExternalInput")
    low = nc.dram_tensor("low", [CC, D2, H2, W2], f32, kind="ExternalOutput")
    high = nc.dram_tensor("high", [CC, 7, D2, H2, W2], f32, kind="ExternalOutput")

    with tile.TileContext(nc) as tc:
        with (
            tc.tile_pool(name="wpool", bufs=1) as wpool,
            tc.tile_pool(name="io", bufs=3) as io,
            tc.tile_pool(name="mid", bufs=2) as mid,
            tc.tile_pool(name="obuf", bufs=2) as obuf,
            tc.tile_pool(name="psum", bufs=8, space="PSUM") as psum,
        ):
            w_sb = wpool.tile([128, 128], f32)
            nc.sync.dma_start(out=w_sb[:, :], in_=wz[:, :])

            for cp in range(NCPAIR):
                for hc in range(NHCHUNK):
                    # ---- load: partition p = dp*4 + i*2 + u, free = (h, w).
                    # dp (32) is the OUTERMOST dram-AP dim: HWDGE assigns
                    # descriptors to SDMA engines by outer-dim index, so an
                    # outer dim of 32 spreads across all 16 engines (outer=2
                    # degenerates to 2 engines at ~50 GB/s).
                    xin = io.tile([128, FREE], f32, tag="xin")
                    src = x[2 * cp:2 * cp + 2, :, hc * HCH:(hc + 1) * HCH, :]
                    src = src.rearrange("c (dp i) h w -> dp i c (h w)", i=2)
                    nc.sync.dma_start(out=xin[:, :], in_=src)

                    # ---- stage X: w butterfly -> xt free = (sx, h, w')
                    xt = mid.tile([128, FREE], f32, tag="xt")
                    xv = xin[:, :].rearrange("p (f two) -> p f two", two=2)
                    xe, xo = xv[:, :, 0], xv[:, :, 1]
                    nc.vector.tensor_add(out=xt[:, 0:FREE // 2], in0=xe, in1=xo)
                    nc.vector.tensor_sub(out=xt[:, FREE // 2:FREE], in0=xe, in1=xo)

                    # ---- stage Y: h butterfly -> yt free = (sy, sx, h2, w')
                    yt = mid.tile([128, FREE], f32, tag="yt")
                    xtv = xt[:, :].rearrange(
                        "p (sx h2 two wp) -> p sx h2 two wp", sx=2, h2=HCH // 2, two=2
                    )
                    ye, yo = xtv[:, :, :, 0, :], xtv[:, :, :, 1, :]
                    ytv = yt[:, :].rearrange(
                        "p (sy sx h2 wp) -> p sy sx h2 wp", sy=2, sx=2, h2=HCH // 2
                    )
                    nc.vector.tensor_add(out=ytv[:, 0], in0=ye, in1=yo)
                    nc.vector.tensor_sub(out=ytv[:, 1], in0=ye, in1=yo)

                    # ---- stage Z: d butterfly + scale, one matmul per 512
                    # cols; ScalarE drains PSUM into a 2-chunk accumulator
                    # (free = (sy, sx, h2pair:32, w')) so store descriptors
                    # reach 8 KiB — descriptor GENERATION costs ~13 ns/desc
                    # on the issuing engine, so fewer/bigger descs keep the
                    # ACT sequencer off the critical path.
                    if hc % 2 == 0:
                        ot = obuf.tile([128, 2 * FREE], f32, tag="ot")
                    for n in range(FREE // 512):
                        ps = psum.tile([128, 512], f32, tag="ps")
                        nc.tensor.matmul(
                            ps[:, :], w_sb[:, :], yt[:, n * 512:(n + 1) * 512],
                            start=True, stop=True,
                        )
                        sy_, sx_, hh = n // 4, (n // 2) % 2, n % 2
                        off = sy_ * 4096 + sx_ * 2048 + (hc % 2) * 1024 + hh * 512
                        nc.scalar.copy(out=ot[:, off:off + 512], in_=ps[:, :])

                    # ---- store after every odd chunk: one DMA per subband,
                    # all on the ACT HWDGE ring -- the SP ring carries ONLY
                    # loads, since a store parked behind a semaphore at the
                    # head of the SP FIFO would block the next tile's load
                    # (head-of-line) and starve the whole pipeline.
                    if hc % 2 == 1:
                        h2lo = (hc // 2) * HCH
                        h2hi = h2lo + HCH
                        for sz in range(2):
                            for sy in range(2):
                                for sx in range(2):
                                    s = sz * 4 + sy * 2 + sx
                                    fo = sy * 4096 + sx * 2048
                                    sb = ot[sz * 64:(sz + 1) * 64, fo:fo + 2048]
                                    if s == 0:
                                        dst = low[2 * cp:2 * cp + 2, :, h2lo:h2hi, :]
                                    else:
                                        dst = high[2 * cp:2 * cp + 2, s - 1, :,
                                                   h2lo:h2hi, :]
                                    dst = dst.rearrange("c dp h w -> dp c (h w)")
                                    nc.scalar.dma_start(out=dst, in_=sb)

    nc.finalize()
    return nc


def _zweights(kernels: np.ndarray) -> np.ndarray:
    """Stage-Z weight: wz[k=(dp,i,u), m=(sz,dp,u)] = tz_sz[i] / 2 on the
    (dp,u) diagonal.  kernels[4*sz, i, 0, 0] = tz_sz[i] * ty0[0] * tx0[0]
    = tz_sz[i]/2 exactly as the reference computed it in fp32."""
    wz = np.zeros((128, 128), np.float32)
    dp = np.arange(32)
    for i in range(2):
        for u in range(2):
            for sz in range(2):
                wz[dp * 4 + i * 2 + u, sz * 64 + dp * 2 + u] = np.float32(
                    kernels[4 * sz, i, 0, 0]
                )
    return wz


def kernel(x, kernels):
    from concourse.bass_utils import run_bass_kernel_spmd

    x = np.asarray(x, dtype=np.float32)
    kernels = np.asarray(kernels, dtype=np.float32)
    assert x.shape == (B, C, D, H, W), x.shape

    if "nc" not in _CACHE:
        _CACHE["nc"] = _build_module()
    nc = _CACHE["nc"]

    wz = _zweights(kernels)
    xf = x.reshape(B * C, D, H, W)
    in_maps = [
        {"x": np.ascontiguousarray(xf[k * CC:(k + 1) * CC]), "wz": wz}
        for k in range(NCORES)
    ]
    res = run_bass_kernel_spmd(nc, in_maps, core_ids=list(range(NCORES)))

    low = np.concatenate([r["low"] for r in res.results], axis=0)
    high = np.concatenate([r["high"] for r in res.results], axis=0)
    low = low.reshape(B, C, D2, H2, W2)
    high = high.reshape(B, C, 7, D2, H2, W2)
    return low, high


# revision 16
# speedup vs baseline: 1.3370x; 1.1050x over previous
"""3D Haar DWT (2x2x2 stride-2) on 8 Trainium2 NeuronCores.

Input  x: (2, 32, 64, 128, 128) fp32, kernels: (8, 2, 2, 2) fp32 (fixed Haar taps).
Output (low, highs): low (2, 32, 32, 64, 64), highs (2, 32, 7, 32, 64, 64).

Sharding: pure data parallel over the 64 (b, c) channel-planes -> 8 channels
per core.  Per core the transform is computed separably:
  - stage X (w butterfly)  : DVE tensor_add/tensor_sub on stride-2 free-dim APs
  - stage Y (h butterfly)  : DVE tensor_add/tensor_sub on free-dim APs
  - stage Z (d butterfly + global 1/(2*sqrt(2)) scale): one 128x128 fp32 matmul
    per 512-wide chunk.  Partition dim is (d-parity, channel-in-pair, d'), so
    the Z weight is the block matrix [[I64, I64], [I64, -I64]] * tz/2.
  - ScalarE copies PSUM->SBUF, then per-subband DMA-out with 4 KB granules.

All HBM traffic is contiguous in >=4 KiB runs (load granule: 16 KiB).
"""

import numpy as np

# Per-core problem geometry (hardcoded; the harness always passes the full
# (2, 32, 64, 128, 128) input).
B, C, D, H, W = 2, 32, 64, 128, 128
NCORES = 8
CC = (B * C) // NCORES          # 8 channel-planes per core
D2, H2, W2 = D // 2, H // 2, W // 2
NCPAIR = CC // 2                # 4 channel pairs per core
NHCHUNK = 4                     # h chunks of 32 lines
HCH = H // NHCHUNK              # 32 h lines per chunk
FREE = HCH * W                  # 4096 fp32 per partition per tile

_CACHE = {}


def _build_module():
    import concourse.bacc as bacc
    import concourse.mybir as mybir
    import concourse.tile as tile

    f32 = mybir.dt.float32
    nc = bacc.Bacc(None, target_bir_lowering=False)

    x = nc.dram_tensor("x", [CC, D, H, W], f32, kind="ExternalInput")
    wz = nc.dram_tensor("wz", [128, 128], mybir.dt.float32r, kind="ExternalInput")
    low = nc.dram_tensor("low", [CC, D2, H2, W2], f32, kind="ExternalOutput")
    high = nc.dram_tensor("high", [CC, 7, D2, H2, W2], f32, kind="ExternalOutput")

    with tile.TileContext(nc) as tc:
        with (
            tc.tile_pool(name="wpool", bufs=1) as wpool,
            tc.tile_pool(name="io", bufs=3) as io,
            tc.tile_pool(name="mid", bufs=2) as mid,
            tc.tile_pool(name="obuf", bufs=2) as obuf,
            tc.tile_pool(name="psum", bufs=2, space="PSUM") as psum,
        ):
            w_sb = wpool.tile([128, 128], mybir.dt.float32r)
            nc.sync.dma_start(out=w_sb[:, :], in_=wz[:, :])

            for cp in range(NCPAIR):
                for hc in range(NHCHUNK):
                    # ---- load: partition p = dp*4 + i*2 + u, free = (h, w).
                    # dp (32) is the OUTERMOST dram-AP dim: HWDGE assigns
                    # descriptors to SDMA engines by outer-dim index, so an
                    # outer dim of 32 spreads across all 16 engines (outer=2
                    # degenerates to 2 engines at ~50 GB/s).
                    xin = io.tile([128, FREE], f32, tag="xin")
                    src = x[2 * cp:2 * cp + 2, :, hc * HCH:(hc + 1) * HCH, :]
                    src = src.rearrange("c (dp i) h w -> dp i c (h w)", i=2)
                    nc.sync.dma_start(out=xin[:, :], in_=src)

                    # ---- stage X: w butterfly -> xt free = (sx, h, w')
                    xt = mid.tile([128, FREE], f32, tag="xt")
                    xv = xin[:, :].rearrange("p (f two) -> p f two", two=2)
                    xe, xo = xv[:, :, 0], xv[:, :, 1]
                    nc.vector.tensor_add(out=xt[:, 0:FREE // 2], in0=xe, in1=xo)
                    nc.vector.tensor_sub(out=xt[:, FREE // 2:FREE], in0=xe, in1=xo)

                    # ---- stage Y: h butterfly -> yt free = (sy, sx, h2, w')
                    yt = mid.tile([128, FREE], mybir.dt.float32r, tag="yt")
                    xtv = xt[:, :].rearrange(
                        "p (sx h2 two wp) -> p sx h2 two wp", sx=2, h2=HCH // 2, two=2
                    )
                    ye, yo = xtv[:, :, :, 0, :], xtv[:, :, :, 1, :]
                    ytv = yt[:, :].rearrange(
                        "p (sy sx h2 wp) -> p sy sx h2 wp", sy=2, sx=2, h2=HCH // 2
                    )
                    nc.vector.tensor_add(out=ytv[:, 0], in0=ye, in1=yo)
                    nc.vector.tensor_sub(out=ytv[:, 1], in0=ye, in1=yo)

                    # ---- stage Z: d butterfly + scale.  float32r matmuls
                    # stream at 1 cycle/col for N>=256 (plain fp32 pays 4) --
                    # same 4-byte data, replicated-precision PE mode.  Four
                    # matmuls fill a 4-bank PSUM tile, then ONE ACTIVATE
                    # copies all 2048 cols to SBUF (amortizes the ~172-cycle
                    # ScalarE fixed cost).
                    if hc % 2 == 0:
                        ot = obuf.tile([128, 2 * FREE], f32, tag="ot")
                    for q in range(FREE // 2048):
                        ps = psum.tile([128, 2048], f32, tag="ps")
                        for k in range(4):
                            n = q * 4 + k
                            nc.tensor.matmul(
                                ps[:, k * 512:(k + 1) * 512],
                                w_sb[:, :],
                                yt[:, n * 512:(n + 1) * 512],
                                start=True, stop=True,
                            )
                        # q = sy; the 2048 cols are (sx:2)x(h2:16)x(w':64).
                        # One 3-dim ACTIVATE writes both sx blocks into ot.
                        src = ps[:, :].rearrange("p (sx f) -> p sx f", sx=2)
                        otv = ot[:, :].rearrange("p (sy sx f) -> p sy sx f",
                                                 sy=2, sx=2)
                        dst = otv[:, q, :, (hc % 2) * 1024:(hc % 2) * 1024 + 1024]
                        nc.scalar.copy(out=dst, in_=src)

                    # ---- store after every odd chunk: subband-batched DMAs
                    # (adjacent s values share one DMA), all on the ACT HWDGE
                    # ring -- the SP ring carries ONLY loads, since a store
                    # parked at the head of the SP FIFO would block the next
                    # tile's load (head-of-line) and starve the pipeline.
                    if hc % 2 == 1:
                        h2lo = (hc // 2) * HCH
                        h2hi = h2lo + HCH
                        # Output partitions are (sz, u, dp): per-channel
                        # batched stores read contiguous partition blocks.
                        # Each DMA keeps the dram AP at 3 dims [dp][s][(h w)]
                        # with outer dim 32 for full SDMA-engine spread.
                        for u in range(2):
                            c = 2 * cp + u
                            dst = low[c, :, h2lo:h2hi, :]
                            dst = dst.rearrange("dp h w -> dp (h w)")
                            nc.scalar.dma_start(
                                out=dst, in_=ot[u * 32:(u + 1) * 32, 0:2048])
                            dst = high[c, 0:3, :, h2lo:h2hi, :]
                            dst = dst.rearrange("s dp h w -> dp s (h w)")
                            nc.scalar.dma_start(
                                out=dst, in_=ot[u * 32:(u + 1) * 32, 2048:8192])
                            dst = high[c, 3:7, :, h2lo:h2hi, :]
                            dst = dst.rearrange("s dp h w -> dp s (h w)")
                            nc.scalar.dma_start(
                                out=dst, in_=ot[64 + u * 32:64 + (u + 1) * 32, 0:8192])

    nc.finalize()
    return nc


def _zweights(kernels: np.ndarray) -> np.ndarray:
    """Stage-Z weight: wz[k=(dp,i,u), m=(sz,u,dp)] = tz_sz[i] / 2 on the
    (dp,u) diagonal.  kernels[4*sz, i, 0, 0] = tz_sz[i] * ty0[0] * tx0[0]
    = tz_sz[i]/2 exactly as the reference computed it in fp32."""
    wz = np.zeros((128, 128), np.float32)
    dp = np.arange(32)
    for i in range(2):
        for u in range(2):
            for sz in range(2):
                wz[dp * 4 + i * 2 + u, sz * 64 + u * 32 + dp] = np.float32(
                    kernels[4 * sz, i, 0, 0]
                )
    return wz


def kernel(x, kernels):
    from concourse.bass_utils import run_bass_kernel_spmd

    x = np.asarray(x, dtype=np.float32)
    kernels = np.asarray(kernels, dtype=np.float32)
    assert x.shape == (B, C, D, H, W), x.shape

    if "nc" not in _CACHE:
        _CACHE["nc"] = _build_module()
    nc = _CACHE["nc"]

    wz = _zweights(kernels)
    xf = x.reshape(B * C, D, H, W)
    in_maps = [
        {"x": np.ascontiguousarray(xf[k * CC:(k + 1) * CC]), "wz": wz}
        for k in range(NCORES)
    ]
    res = run_bass_kernel_spmd(nc, in_maps, core_ids=list(range(NCORES)))

    low = np.concatenate([r["low"] for r in res.results], axis=0)
    high = np.concatenate([r["high"] for r in res.results], axis=0)
    low = low.reshape(B, C, D2, H2, W2)
    high = high.reshape(B, C, 7, D2, H2, W2)
    return low, high
